# revision 32
# baseline (speedup 1.0000x reference)
"""Trainium2 Bass kernel for the RNN-T JointNetwork problem.

  enc = h_enc @ W_enc + b_enc            (B,T,1,J)
  dec = h_dec @ W_dec                    (B,1,U,J)
  z   = tanh(enc + dec)                  (B,T,U,J)
  out = z @ W_out + b_out                (B,T,U,V)

Shapes: B=4, T=256, U=64, D=J=V=512, fp32 in/out.

Sharding: 8 cores, data parallel over (B x T/2): core c handles batch
b = c//2 and t-half th = c%2 (128 t values). Params replicated.

Design (v3, 80.9us vs the 83.2us baseline; measured floors: preamble
~7.3us fixed, input wire ~230GB/s -> all inputs in SBUF ~15.3us, PE
roofline 54.6us for the 131072 main matmul columns):
  - TRANSPOSED OUTPUT (outT[v, row]): W_out chunks stationary, zT
    moving, b_out per-partition; PE runs the main matmuls at roofline
    cadence (~216ns per 512-wide matmul), zero >300ns gaps in steady.
  - U-MAJOR z rows within each t-group: row = u*tg + t_local, with
    dec pre-replicated over t (dec_repU) so both zpre-add operands are
    innermost-step-1 bf16 APs (DVE 2x mode).  Host un-permutes.
  - FILL: inputs land as per-chunk DMAs serialized on the sync queue
    in consumer order (dec chunks -> enc chunks -> wout jc-quarters,
    biases on scalar), so every setup matmul fires as its own chunk
    lands and the first out-block's jc-outer matmul order matches the
    wout quarter arrivals.  dec evacs on DVE; enc evacs split DVE/ACT;
    per-jc tanh for groups 0/1 interleaved with repU replication.
  - STEADY: software-pipelined emission order - group g+1's adds+tanh
    are emitted BEFORE group g's output matmuls+evacs, so the in-order
    DVE queue never delays the z-path behind the evac backlog, and the
    in-order ACT queue runs tanh(g+1) before evacs of g.  One PSUM
    pool (8 banks, one shared tag) covers setup + all out blocks.
    Evacs split 3:5 ACT:DVE.
  - DRAIN: group taper [...8,4,2,2]; the last two blocks force
    ACT/DVE evac alternation and leave as per-half DMAs on
    sync+gpsimd as soon as each evac pair completes.
Known-bad variants (measured): GpSimd tensor ops alongside DVE
perf-mode work (lockstep stall); stride-0-innermost bf16 adds; K=1
bias matmuls on PE; fp8 DoubleRow main matmuls (z-side e4m3
quantization alone gives 2.4e-2 l2 rel err > 2e-2 gate); 4:4 evac
split (ACT-bound -> PE starves at group boundaries); consumers
emitted before their producers race (Tile deps are emission-ordered);
parallel input DMA queues (wire-bound, ~230GB/s regardless -- only
consumer-ordered serialization helps).
"""

import numpy as np

B, T, U = 4, 256, 64
D, J, V = 512, 512, 512
NCORES = 8
TH = T // 2          # t's per core = 128
KC = 4               # 512/128 contraction chunks
VQ = 4               # v-quarters (output partition chunks)
MAXTG = 16

GROUP_T = [4, 12] + [16] * 6 + [8, 4, 2, 2]
assert sum(GROUP_T) == TH

SPLIT_WAITS = True   # walrus single-wait workaround (off for CoreSim runs)
TANH_SPLIT = set()       # loop groups whose tanh is emitted per-jc

_compiled = None

# bdec: per chunk k: hdect_k | wdec_k  (bf16) -- lands first, smallest
BDEC_K = U + J                   # 576 cols per chunk
# benc: per chunk k: henct_k | wenc_k  (bf16)
BENC_K = TH + J                  # 640 cols per chunk
# blob2: wout (bf16), stationary chunks
B2_COLS = KC * V
# blob3: benc | boutp  (fp32)
B3_COLS = 2 * KC


def _build():
    import concourse.bass as bass
    import concourse.tile as tile
    from concourse import mybir

    fp32 = mybir.dt.float32
    bf16 = mybir.dt.bfloat16
    AF = mybir.ActivationFunctionType

    nc = bass.Bass()

    bdec_d = nc.declare_dram_parameter("bdec", [128, KC * BDEC_K], bf16, isOutput=False)
    benc_d = nc.declare_dram_parameter("benc", [128, KC * BENC_K], bf16, isOutput=False)
    b2_d = nc.declare_dram_parameter("b2", [128, B2_COLS], bf16, isOutput=False)
    b3_d = nc.declare_dram_parameter("b3", [128, B3_COLS], fp32, isOutput=False)
    out = nc.declare_dram_parameter("out", [V, TH * U], bf16, isOutput=True)

    with tile.TileContext(nc) as tc:
        with (
            tc.tile_pool(name="const", bufs=1) as const,
            tc.tile_pool(name="zpre", bufs=5) as zpre_pool,
            tc.tile_pool(name="zt", bufs=5) as zt_pool,
            tc.tile_pool(name="outs", bufs=8) as outs_pool,
            tc.tile_pool(name="ps", bufs=8, space="PSUM") as ps,
        ):
            # ---- input DMAs: per-chunk transfers serialized on the sync
            # queue in consumer order (dec chunks -> enc chunks -> wout
            # jc-quarters), so every setup matmul can fire as soon as its
            # own chunk lands instead of waiting for a whole blob.  The
            # wire is the constraint (~180GB/s effective), so a single
            # priority-ordered queue beats parallel queues.  Biases
            # (tiny) ride the scalar queue. ----
            bdec = const.tile([128, KC * BDEC_K], bf16, tag="bdec")
            for k in range(KC):
                nc.sync.dma_start(
                    bdec[:, k * BDEC_K:(k + 1) * BDEC_K],
                    bdec_d[:, k * BDEC_K:(k + 1) * BDEC_K],
                )
            benc = const.tile([128, KC * BENC_K], bf16, tag="benc")
            for k in range(KC):
                nc.sync.dma_start(
                    benc[:, k * BENC_K:(k + 1) * BENC_K],
                    benc_d[:, k * BENC_K:(k + 1) * BENC_K],
                )
            b2 = const.tile([128, B2_COLS], bf16, tag="b2")
            for jc in range(KC):
                nc.sync.dma_start(
                    b2[:, jc * V:(jc + 1) * V], b2_d[:, jc * V:(jc + 1) * V]
                )
            b3 = const.tile([128, B3_COLS], fp32, tag="b3")
            nc.scalar.dma_start(b3[:], b3_d[:])

            # ---- ACT table warmup ----
            warm = const.tile([1, 2], fp32, tag="warm")
            nc.vector.memset(warm[:], 0.0)
            nc.scalar.activation(warm[0:1, 1:2], warm[0:1, 0:1], AF.Tanh)

            def hdect_k(k):
                c = k * BDEC_K
                return bdec[:, c:c + U]

            def wdec_kj(k, jc):
                c = k * BDEC_K + U + jc * 128
                return bdec[:, c:c + 128]

            def henct_k(k):
                c = k * BENC_K
                return benc[:, c:c + TH]

            def wenc_kj(k, jc):
                c = k * BENC_K + TH + jc * 128
                return benc[:, c:c + 128]

            def wout_jv(jc, vq):
                c = (jc * VQ + vq) * 128
                return b2[:, c:c + 128]

            benc_s = b3[:, 0:KC]
            boutp_s = b3[:, KC:2 * KC]

            # ---- encT / decT setup matmuls.  k-OUTER accumulation into
            # two wide PSUM tiles (slots of the shared 8-bank pool); dec
            # runs before enc within each half-blob so the dec->repU->add
            # chain starts as early as possible.  Per-region stop flags on
            # the k=3 matmuls let each jc's evac fire as soon as its own
            # region is complete. ----
            pe_all = ps.tile([128, 512], fp32, tag="po")
            pd_all = ps.tile([128, 512], fp32, tag="po")
            for k in range(KC):
                for jc in range(KC):
                    nc.tensor.matmul(
                        pd_all[:, jc * U:(jc + 1) * U],
                        wdec_kj(k, jc),
                        hdect_k(k),
                        start=(k == 0 and jc == 0),
                        stop=(k == KC - 1),
                        skip_group_check=True,
                    )
            for k in range(KC):
                for jc in range(KC):
                    nc.tensor.matmul(
                        pe_all[:, jc * TH:(jc + 1) * TH],
                        wenc_kj(k, jc),
                        henct_k(k),
                        start=(k == 0 and jc == 0),
                        stop=(k == KC - 1),
                        skip_group_check=True,
                    )

            # ---- dec evacs on DVE (frees ACT for enc evacs), then
            # dec_repU[jc][j, u, t] = decT[jc][j, u] replicated over t
            # (DVE copies), interleaved with group 0's adds; enc evacs on
            # ACT interleaved with group 0's per-jc tanh so the first
            # main matmul starts as soon as zt jc0 exists. ----
            decT_s = []
            for jc in range(KC):
                dt_ = const.tile([128, U], bf16, tag=f"decT{jc}")
                nc.vector.tensor_copy(dt_[:], pd_all[:, jc * U:(jc + 1) * U])
                decT_s.append(dt_)

            dr_s = []
            for jc in range(KC):
                dr = const.tile([128, U * MAXTG], bf16, tag=f"drepU{jc}")
                dr_s.append(dr)
            encT_s = []
            for jc in range(KC):
                et = const.tile([128, TH], bf16, tag=f"encT{jc}")
                encT_s.append(et)

            def emit_repu(jc):
                dr3 = dr_s[jc][:].rearrange("p (u t) -> p u t", u=U)
                src = (
                    decT_s[jc][:]
                    .rearrange("p (u x) -> p u x", x=1)
                    .to_broadcast([128, U, MAXTG])
                )
                nc.vector.tensor_copy(dr3, src)

            def emit_encv(jc):
                nc.scalar.activation(
                    encT_s[jc][:], pe_all[:, jc * TH:(jc + 1) * TH],
                    AF.Identity, bias=benc_s[:, jc:jc + 1],
                )

            def emit_encv_dve(jc):
                nc.vector.tensor_scalar_add(
                    encT_s[jc][:], pe_all[:, jc * TH:(jc + 1) * TH],
                    benc_s[:, jc:jc + 1],
                )

            # ---- main loop, software-pipelined emission: the z-path of
            # group g+1 (adds + tanh) is emitted BEFORE the out-path of
            # group g (matmuls + evacs + dma), so in-order engine queues
            # never stall the z-path behind the evac backlog. ----
            n_blocks_total = sum(
                (tg * U) // 512 + (1 if (tg * U) % 512 else 0) for tg in GROUP_T
            )
            state = {"evac_ctr": 0, "blocks_done": 0}

            def emit_evac(po, ob, vq, w, force=None):
                obs = ob[:, vq * 512:vq * 512 + w]
                if force is not None:
                    on_act = force == "act"
                else:
                    on_act = (state["evac_ctr"] * 3) % 8 < 3
                if on_act:
                    nc.scalar.activation(
                        obs, po[:, 0:w], AF.Identity,
                        bias=boutp_s[:, vq:vq + 1],
                    )
                else:
                    nc.vector.tensor_scalar_add(
                        obs, po[:, 0:w], boutp_s[:, vq:vq + 1]
                    )
                state["evac_ctr"] += 1

            def emit_out(tg, col0, zt):
                gw = tg * U
                widths = [512] * (gw // 512)
                if gw % 512:
                    widths.append(gw % 512)
                rc = 0
                for w in widths:
                    state["blocks_done"] += 1
                    is_first = state["blocks_done"] == 1
                    is_tail = state["blocks_done"] >= n_blocks_total - 1
                    c0 = col0 + rc
                    ob = outs_pool.tile([128, VQ * 512], bf16, tag="ob")
                    pos = []
                    for vq in range(VQ):
                        po = ps.tile([128, 512], fp32, tag="po")
                        pos.append(po)
                    if is_first:
                        # jc-outer: matches the arrival order of the wout
                        # jc-quarter DMAs during the fill
                        order = [(jc, vq) for jc in range(KC) for vq in range(VQ)]
                    else:
                        order = [(jc, vq) for vq in range(VQ) for jc in range(KC)]

                    def half_dma(eng, lohi):
                        eng.dma_start(
                            out[:, c0:c0 + w]
                            .rearrange("(v p) c -> p v c", v=VQ)[:, lohi:lohi + 2],
                            ob[:]
                            .rearrange("p (v c) -> p v c", v=VQ)[:, lohi:lohi + 2, 0:w],
                        )

                    for jc, vq in order:
                        nc.tensor.matmul(
                            pos[vq][:, 0:w],
                            wout_jv(jc, vq),
                            zt[:, jc * gw + rc:jc * gw + rc + w],
                            start=(jc == 0),
                            stop=(jc == KC - 1),
                        )
                        if not is_first and jc == KC - 1:
                            if is_tail:
                                # tail blocks: forced ACT/DVE alternation and
                                # per-half DMAs on both queues so the final
                                # transfers start as early as possible
                                emit_evac(pos[vq], ob, vq, w,
                                          force="act" if vq % 2 == 0 else "dve")
                                if vq == 1:
                                    half_dma(nc.sync, 0)
                                elif vq == 3:
                                    half_dma(nc.gpsimd, 2)
                            else:
                                emit_evac(pos[vq], ob, vq, w)
                    if is_first:
                        for vq in range(VQ):
                            emit_evac(pos[vq], ob, vq, w)
                    if not is_tail:
                        nc.sync.dma_start(
                            out[:, c0:c0 + w].rearrange("(v p) c -> p v c", v=VQ),
                            ob[:].rearrange("p (v c) -> p v c", v=VQ)[:, :, 0:w],
                        )
                    rc += w

            def emit_add(zp, jc, tg, col0):
                gw = tg * U
                zps = zp[:, jc * gw:(jc + 1) * gw]
                zp3 = zps.rearrange("p (u t) -> p u t", u=U)
                d3 = (
                    dr_s[jc][:]
                    .rearrange("p (u t) -> p u t", u=U)[:, :, 0:tg]
                )
                e3 = (
                    encT_s[jc][:, col0 // U:col0 // U + tg]
                    .rearrange("p (x t) -> p x t", x=1)
                    .to_broadcast([128, U, tg])
                )
                nc.vector.tensor_add(zp3, d3, e3)

            def emit_tanh_jc(zt, zp, jc, gw):
                nc.scalar.activation(
                    zt[:, jc * gw:(jc + 1) * gw],
                    zp[:, jc * gw:(jc + 1) * gw], AF.Tanh,
                )

            # ---- fill: z-paths of groups 0 and 1, interleaved with the
            # repU replications (DVE) and enc evacs (ACT) so neither
            # engine stalls on the other's outputs. ----
            tg0, tg1 = GROUP_T[0], GROUP_T[1]
            zp0 = zpre_pool.tile([128, KC * MAXTG * U], bf16, tag="zp")
            zt0 = zt_pool.tile([128, KC * MAXTG * U], bf16, tag="zt")
            zp1 = zpre_pool.tile([128, KC * MAXTG * U], bf16, tag="zp")
            zt1 = zt_pool.tile([128, KC * MAXTG * U], bf16, tag="zt")
            # enc evacs jc0/jc1 on DVE (free after repU), jc2/jc3 on ACT;
            # per-jc tanh interleaves so the first matmuls start early
            emit_repu(0)
            emit_repu(1)
            emit_repu(2)
            emit_repu(3)
            emit_encv(2)
            emit_encv_dve(0)
            emit_add(zp0, 0, tg0, 0)
            emit_tanh_jc(zt0, zp0, 0, tg0 * U)
            emit_encv_dve(1)
            emit_add(zp0, 1, tg0, 0)
            emit_encv(3)
            emit_tanh_jc(zt0, zp0, 1, tg0 * U)
            emit_add(zp0, 2, tg0, 0)
            emit_add(zp0, 3, tg0, 0)
            emit_tanh_jc(zt0, zp0, 2, tg0 * U)
            emit_tanh_jc(zt0, zp0, 3, tg0 * U)
            c1 = tg0 * U
            for jc in range(KC):
                emit_add(zp1, jc, tg1, c1)
                emit_tanh_jc(zt1, zp1, jc, tg1 * U)

            # ---- pipelined main loop: z(g) is emitted before out(g-2)
            # and out(g-1), keeping the in-order DVE/ACT queues' z-path
            # ops ahead of the evac backlog. ----
            queue = [(tg0, 0, zt0), (tg1, c1, zt1)]
            col0 = c1 + tg1 * U
            for g in range(2, len(GROUP_T)):
                tg = GROUP_T[g]
                gw = tg * U
                zp = zpre_pool.tile([128, KC * MAXTG * U], bf16, tag="zp")
                for jc in range(KC):
                    emit_add(zp, jc, tg, col0)
                zt = zt_pool.tile([128, KC * MAXTG * U], bf16, tag="zt")
                if g in TANH_SPLIT:
                    for jc in range(KC):
                        emit_tanh_jc(zt, zp, jc, gw)
                else:
                    nc.scalar.activation(
                        zt[:, :KC * gw], zp[:, :KC * gw], AF.Tanh
                    )
                emit_out(*queue.pop(0))
                queue.append((tg, col0, zt))
                col0 += gw
            for p in queue:
                emit_out(*p)

    if SPLIT_WAITS:
        _split_multi_waits(nc)
    return nc


_COMPUTE_OPS = {
    "Matmult", "Ldweights", "TensorTensor", "TensorCopy", "TensorScalarPtr",
    "Activation", "TensorReduce", "Memset", "ScalarTensorTensor",
    "TensorScalar", "DMACopy", "Drain", "EventSemaphore",
}


def _split_multi_waits(nc):
    """walrus codegen in this container allows a single sync-wait command
    per TPB compute instruction; Tile emits several.  Hoist all but one
    wait onto standalone EventSemaphore instructions placed just before
    the offending instruction (same engine, so semantics are identical).
    """
    from concourse import mybir

    ctr = [0]
    for fn in nc.m.functions:
        for blk in fn.blocks:
            insts = blk.instructions
            out = []
            for inst in insts:
                si = getattr(inst, "sync_info", None)
                ow = list(si.on_wait) if si and si.on_wait else []
                if (
                    len(ow) > 1
                    and getattr(inst, "opcode", None) in _COMPUTE_OPS
                ):
                    for w in ow[:-1]:
                        ctr[0] += 1
                        ev = mybir.InstEventSemaphore(
                            name=f"WS-{ctr[0]}-{inst.name}",
                            ins=[],
                            outs=[],
                            sync_info=mybir.SyncInfo(
                                on_wait=[w], on_update=[]
                            ),
                        )
                        ev.engine = inst.engine
                        out.append(ev)
                    inst.sync_info = mybir.SyncInfo(
                        on_wait=[ow[-1]], on_update=list(si.on_update or [])
                    )
                out.append(inst)
            blk.instructions = out


def _get_compiled():
    global _compiled
    if _compiled is None:
        _compiled = _build()
    return _compiled


def _chunk_rows(mat, ncols):
    """[D, N] (contraction-major) -> [128, KC*N] f32, chunk-interleaved:
    out[p, k*N + n] = mat[k*128 + p, n]"""
    m = np.asarray(mat, dtype=np.float32).reshape(KC, 128, ncols)
    return m.transpose(1, 0, 2).reshape(128, KC * ncols)


def kernel(h_enc, h_dec, W_enc, b_enc, W_dec, W_out, b_out, **_):
    import ml_dtypes

    nc = _get_compiled()
    from concourse.bass_utils import run_bass_kernel_spmd

    bf16 = ml_dtypes.bfloat16
    h_enc = np.asarray(h_enc, dtype=np.float32)
    h_dec = np.asarray(h_dec, dtype=np.float32)

    # per-chunk [128, N] views, chunk-major lists
    wenc_k = _chunk_rows(W_enc, J).reshape(128, KC, J)
    wdec_k = _chunk_rows(W_dec, J).reshape(128, KC, J)
    blob2 = np.ascontiguousarray(
        np.asarray(W_out, dtype=np.float32)
        .reshape(KC, 128, VQ, 128)
        .transpose(1, 0, 2, 3)
        .reshape(128, KC * V)
        .astype(bf16)
    )
    blob3 = np.ascontiguousarray(
        np.concatenate(
            [
                np.asarray(b_enc, dtype=np.float32).reshape(KC, 128).T,
                np.asarray(b_out, dtype=np.float32).reshape(VQ, 128).T,
            ],
            axis=1,
        )
    )

    bdec_b = {}
    for b in range(B):
        hdect_k = _chunk_rows(h_dec[b, 0, :, :].T, U).reshape(128, KC, U)
        parts = []
        for k in range(KC):
            parts.extend([hdect_k[:, k], wdec_k[:, k]])
        bdec_b[b] = np.ascontiguousarray(
            np.concatenate(parts, axis=1).astype(bf16)
        )
    in_maps = []
    for c in range(NCORES):
        b, th = c // 2, c % 2
        henct_k = _chunk_rows(
            h_enc[b, th * TH:(th + 1) * TH, 0, :].T, TH
        ).reshape(128, KC, TH)
        parts = []
        for k in range(KC):
            parts.extend([henct_k[:, k], wenc_k[:, k]])
        benc = np.ascontiguousarray(np.concatenate(parts, axis=1).astype(bf16))
        in_maps.append(
            {"bdec": bdec_b[b], "benc": benc, "b2": blob2, "b3": blob3}
        )

    global _last_in_maps
    _last_in_maps = in_maps
    res = run_bass_kernel_spmd(nc, in_maps, list(range(NCORES)))

    out_full = np.empty((B, T, U, V), dtype=np.float32)
    for c in range(NCORES):
        b, th = c // 2, c % 2
        outT = np.asarray(res.results[c]["out"]).astype(np.float32)  # [V, 8192]
        t0 = th * TH
        col0 = 0
        for tg in GROUP_T:
            gw = tg * U
            blk = outT[:, col0:col0 + gw].reshape(V, U, tg)
            # [v, u, t] -> [t, u, v]
            out_full[b, t0:t0 + tg] = blk.transpose(2, 1, 0)
            t0 += tg
            col0 += gw
    return out_full
